# revision 1
# baseline (speedup 1.0000x reference)
"""AdaptiveGeometryAttention Trainium2 kernel (8 NeuronCores).

Sharding: core c handles batch b = c//4 and head group hg = c%4 (4 of 16 heads).
Each core computes its heads' attention and a partial out-projection (T, E);
the host sums the 4 partials per batch.

Key algebraic restructurings vs the reference:
  - The Lorentz inner product -<q_hyp, k_hyp>_L is a single K=66 matmul over
    augmented vectors [t, gf*z0, -/+gf*q_d].
  - arccosh(m)^2 ~= A*tanh(sf*(m-1)+bf) + a*(m-1) + const. The constant and
    every per-query additive term cancel in softmax, so they are dropped.
    The a*mdot linear term and the (1-alpha)/8 euclid scale are folded into a
    second accumulated matmul (per-query column scaling of q-side operands).
  - Softmax denominators come from a ones-column appended to V in the
    attn @ v matmul; spike masking and 1/Z fold into one per-query scale
    applied to y^T before the out-projection.
All matmuls run as float32r (full-rate fp32 PE mode).
"""

import sys
import contextlib

sys.path.insert(0, "/opt/trn_rl_repo")

import numpy as np

B, T, E, H = 2, 1024, 1024, 16
D = 64
NCORES = 8
HPC = 4  # heads per core
KB = 9  # K blocks over E+1 (bias row)

# arccosh(1+x)^2 fit on x in [0, 2.2]
A_FIT = 54.32641203
S_FIT = 0.28607594936708863
B_FIT = 2.0
A_LIN = 0.8910533
# sqrt(1+w) deg-3 fit on w in [0, 0.95]
SQ3, SQ2, SQ1, SQ0 = 0.02492195, -0.10732602, 0.49672154, 1.00014421

# host-derived scalars, set by kernel() before _build()
_S2 = _S2M2 = _SS = _THR = _BADJ = 0.0


def _build():
    from concourse import bass, mybir, tile, bacc

    F32 = mybir.dt.float32
    F32R = mybir.dt.float32r
    AF = mybir.ActivationFunctionType
    OP = mybir.AluOpType

    def r(ap):
        return ap.bitcast(F32R)

    nc = bacc.Bacc()

    xT = nc.declare_dram_parameter("xT", [E + 1, T], F32R, isOutput=False)
    wqk = nc.declare_dram_parameter("wqk", [E + 1, 512], F32R, isOutput=False)
    wv = nc.declare_dram_parameter("wv", [E + 1, 256], F32R, isOutput=False)
    wai = nc.declare_dram_parameter("wai", [E + 1, 5], F32R, isOutput=False)
    wo = nc.declare_dram_parameter("wo", [256, E], F32R, isOutput=False)
    onesel = nc.declare_dram_parameter("onesel", [128, 4, 16], F32R, isOutput=False)
    cst = nc.declare_dram_parameter("cst", [5, 1], F32, isOutput=False)
    csta = nc.declare_dram_parameter("csta", [4, 1], F32, isOutput=False)
    cstl = nc.declare_dram_parameter("cstl", [4, 1], F32, isOutput=False)
    tri = nc.declare_dram_parameter("tri", [128, 128], F32, isOutput=False)
    selb8 = nc.declare_dram_parameter("selb8", [8, 8 * 128], F32R, isOutput=False)
    selb4 = nc.declare_dram_parameter("selb4", [4, 12 * 128], F32R, isOutput=False)
    out = nc.declare_dram_parameter("out", [T, E], F32, isOutput=True)

    with tile.TileContext(nc) as tc:
        ctx = contextlib.ExitStack()
        with ctx:
            main = ctx.enter_context(tc.tile_pool(name="main", bufs=1))

            # ---- persistent inputs ----
            tWO = main.tile([128, 2, E], F32R)
            for g in range(2):
                nc.sync.dma_start(out=tWO[:, g, :], in_=wo[g * 128:(g + 1) * 128, :])
            tSEL = main.tile([128, 4, 16], F32R)
            nc.sync.dma_start(out=tSEL[:], in_=onesel[:])
            tCST = main.tile([5, 1], F32)
            nc.sync.dma_start(out=tCST[:], in_=cst[:])
            tCA = main.tile([4, 1], F32)
            nc.sync.dma_start(out=tCA[:], in_=csta[:])
            tCL = main.tile([4, 1], F32)
            nc.sync.dma_start(out=tCL[:], in_=cstl[:])
            tTRI = main.tile([128, 128], F32)
            nc.sync.dma_start(out=tTRI[:], in_=tri[:])
            tS8 = main.tile([8, 8 * 128], F32R)
            nc.sync.dma_start(out=tS8[:], in_=selb8[:])
            tS4 = main.tile([4, 12 * 128], F32R)
            nc.sync.dma_start(out=tS4[:], in_=selb4[:])
            tONEf = main.tile([1, 128], F32)
            nc.vector.memset(tONEf[:], 1.0)
            tONE = main.tile([1, 128], F32R)
            nc.vector.tensor_copy(out=tONE[:], in_=tONEf[:])
            tBADJ = main.tile([128, 1], F32)
            nc.vector.memset(tBADJ[:], _BADJ)

            tQ = [main.tile([64, T], F32R, name=f"tQ{h}", tag=f"q{h}")
                  for h in range(HPC)]
            tK = [main.tile([64, T], F32R, name=f"tK{h}", tag=f"k{h}")
                  for h in range(HPC)]
            tV = main.tile([128, 8, HPC, 65], F32R)
            RW = main.tile([8, 7, T], F32)  # row scratch (slot-reused)
            Pgf = main.tile([8, T], F32R)   # gf rows (q0,k0,q1,k1,...)
            Ptim = main.tile([8, T], F32R)  # time rows
            tA = main.tile([5, T], F32)
            beta = main.tile([4, T], F32R)
            gA = main.tile([4, T], F32R)
            ag = main.tile([4, T], F32R)
            spk = main.tile([1, T], F32)
            tY0 = main.tile([128, T], F32R)
            tY1 = main.tile([128, T], F32R)
            tYL = [tY0, tY1]

            # ================= proj + row phase =================
            with tc.tile_pool(name="pin", bufs=1) as pin, \
                 tc.tile_pool(name="ppj", bufs=1, space="PSUM") as ppj, \
                 tc.tile_pool(name="ppj2", bufs=1, space="PSUM") as ppj2:
                tXT = pin.tile([128, KB, T], F32R)
                tWQK = pin.tile([128, KB, 512], F32R)
                for k in range(8):
                    nc.sync.dma_start(out=tWQK[:, k, :],
                                      in_=wqk[k * 128:(k + 1) * 128, :])
                    nc.gpsimd.dma_start(out=tXT[:, k, :],
                                        in_=xT[k * 128:(k + 1) * 128, :])
                nc.sync.dma_start(out=tWQK[0:1, 8, :], in_=wqk[E:E + 1, :])
                nc.gpsimd.dma_start(out=tXT[0:1, 8, :], in_=xT[E:E + 1, :])
                tWV = pin.tile([128, KB, 256], F32R)
                for k in range(8):
                    nc.sync.dma_start(out=tWV[:, k, :],
                                        in_=wv[k * 128:(k + 1) * 128, :])
                nc.sync.dma_start(out=tWV[0:1, 8, :], in_=wv[E:E + 1, :])
                tWAI = pin.tile([128, KB, 5], F32R)
                for k in range(8):
                    nc.sync.dma_start(out=tWAI[:, k, :],
                                        in_=wai[k * 128:(k + 1) * 128, :])
                nc.sync.dma_start(out=tWAI[0:1, 8, :], in_=wai[E:E + 1, :])
                psN2 = ppj2.tile([8, T], F32, tag="norms")
                psZ2 = ppj2.tile([8, T], F32, tag="normz")
                psA = ppj2.tile([5, T], F32, tag="alpha")
                for h in range(HPC):
                    ps = ppj.tile([128, T], F32, tag="psqk")
                    for k in range(KB):
                        kr = 128 if k < 8 else 1
                        for n in range(2):
                            nc.tensor.matmul(
                                ps[:, n * 512:(n + 1) * 512],
                                r(tWQK[0:kr, k, h * 128:(h + 1) * 128]),
                                r(tXT[0:kr, k, n * 512:(n + 1) * 512]),
                                start=(k == 0), stop=(k == KB - 1),
                            )
                    sq = pin.tile([128, T], F32R, tag="sq")
                    nc.scalar.activation(out=sq[:], in_=ps[:], func=AF.Square)
                    nc.vector.tensor_copy(out=tQ[h][:], in_=ps[0:64, :])
                    nc.scalar.copy(out=tK[h][:], in_=ps[64:128, :])
                    for n in range(2):
                        nc.tensor.matmul(
                            psN2[:, n * 512:(n + 1) * 512],
                            r(tSEL[:, h, 0:8]),
                            r(sq[:, n * 512:(n + 1) * 512]),
                            start=(h == 0), stop=(h == HPC - 1),
                        )
                        nc.tensor.matmul(
                            psZ2[:, n * 512:(n + 1) * 512],
                            r(tSEL[:, h, 8:16]),
                            r(sq[:, n * 512:(n + 1) * 512]),
                            start=(h == 0), stop=(h == HPC - 1),
                        )

                tVonef = pin.tile([128, 32], F32)
                nc.vector.memset(tVonef[:], 1.0)
                nc.vector.tensor_copy(out=tV[:, :, :, 64:65], in_=tVonef[:])
                for m in range(8):
                    psv = ppj.tile([128, 256], F32, tag="psqk")
                    for k in range(KB):
                        kr = 128 if k < 8 else 1
                        nc.tensor.matmul(
                            psv[:],
                            r(tXT[0:kr, k, m * 128:(m + 1) * 128]),
                            r(tWV[0:kr, k, :]),
                            start=(k == 0), stop=(k == KB - 1),
                        )
                    nc.scalar.copy(out=tV[:, m, :, 0:64],
                                   in_=psv[:].rearrange("p (h d) -> p h d", h=HPC))

                for k in range(KB):
                    kr = 128 if k < 8 else 1
                    for n in range(2):
                        nc.tensor.matmul(
                            psA[:, n * 512:(n + 1) * 512],
                            r(tWAI[0:kr, k, :]),
                            r(tXT[0:kr, k, n * 512:(n + 1) * 512]),
                            start=(k == 0), stop=(k == KB - 1),
                        )

                # ---- row quantities (slots sA..sF reused) ----
                gf = Pgf[0:8, :]
                tim = Ptim[0:8, :]
                for cc in range(2):
                    cl = slice(cc * 512, (cc + 1) * 512)
                    sA = RW[0:8, 0, cl]
                    sB = RW[0:8, 1, cl]
                    sC = RW[0:8, 2, cl]
                    sD = RW[0:8, 3, cl]
                    sE = RW[0:8, 4, cl]
                    sF = RW[0:8, 5, cl]
                    gfc = Pgf[0:8, cl]
                    timc = Ptim[0:8, cl]
                    nc.vector.tensor_scalar_max(sA, psN2[0:8, cl], 1e-24)  # n2
                    nc.vector.reciprocal(sB, sA)                           # 1/n2
                    nc.vector.tensor_mul(sC, psZ2[0:8, cl], sB)            # q2n
                    nc.vector.tensor_scalar(sA, sC, _S2M2, _S2, op0=OP.mult, op1=OP.add)
                    nc.vector.tensor_scalar_max(sA, sA, 1e-8)             # y = nu^2
                    nc.scalar.activation(out=sE, in_=sB, func=AF.Sqrt)    # invn
                    # f = sinh(nu)/nu = 1 + y/6 + y^2/120 + y^3/5040
                    nc.vector.tensor_scalar(sD, sA, 1.0 / 5040.0, 1.0 / 120.0,
                                            op0=OP.mult, op1=OP.add)
                    nc.vector.tensor_mul(sD, sD, sA)
                    nc.vector.scalar_tensor_tensor(out=sD, in0=sD, scalar=1.0 / 6.0,
                                                   in1=sA, op0=OP.add, op1=OP.mult)
                    nc.vector.tensor_scalar_add(sA, sD, 1.0)              # f
                    nc.vector.scalar_tensor_tensor(out=gfc, in0=sE, scalar=_SS, in1=sA,
                                                   op0=OP.mult, op1=OP.mult)  # gf
                    nc.vector.tensor_scalar(sB, sC, -_S2, _S2, op0=OP.mult, op1=OP.add)
                    nc.vector.tensor_mul(sC, sA, sA)                      # f^2
                    nc.vector.tensor_mul(sB, sC, sB)                      # w
                    nc.vector.tensor_scalar(sC, sB, SQ3, SQ2, op0=OP.mult, op1=OP.add)
                    nc.vector.tensor_mul(sC, sC, sB)
                    nc.vector.scalar_tensor_tensor(out=sC, in0=sC, scalar=SQ1, in1=sB,
                                                   op0=OP.add, op1=OP.mult)
                    nc.vector.tensor_scalar_add(timc, sC, SQ0)            # time

                nc.scalar.activation(out=tA[:], in_=psA[:], func=AF.Tanh,
                                     scale=0.5, bias=tCST[0:5, :])
                tC625 = main.tile([4, 1], F32)
                nc.vector.memset(tC625[:], 0.0625)
                nc.scalar.activation(out=beta[:], in_=tA[0:4, :], func=AF.Identity,
                                     scale=-0.0625, bias=tC625[:])
                nc.scalar.activation(out=gA[:], in_=tA[0:4, :], func=AF.Identity,
                                     scale=tCA[:], bias=tCA[:])
                nc.scalar.activation(out=ag[:], in_=tA[0:4, :], func=AF.Identity,
                                     scale=tCL[:], bias=tCL[:])
                SPK5 = main.tile([5, T], F32)
                nc.vector.tensor_scalar(SPK5[:], psA[0:5, :], _THR, None, op0=OP.is_gt)
                nc.sync.dma_start(out=spk[:], in_=SPK5[4:5, :])

            # ================= per-head attention =================
            with tc.tile_pool(name="hp", bufs=2) as hp, \
                 tc.tile_pool(name="pph", bufs=2, space="PSUM") as pph, \
                 tc.tile_pool(name="pph1", bufs=1, space="PSUM") as pph1:
                for h in range(HPC):
                    qrow = 2 * h
                    krow = 2 * h + 1
                    QH = hp.tile([66, T], F32R, tag="QH")
                    KH = hp.tile([66, T], F32R, tag="KH")
                    gf8 = gf
                    psb = pph1.tile([128, T], F32, tag="psb")
                    for n in range(2):
                        sl = slice(n * 512, (n + 1) * 512)
                        nc.tensor.matmul(psb[0:64, sl],
                                         r(tS8[:, qrow * 128:(qrow + 1) * 128][:, 0:64]),
                                         r(gf8[:, sl]), start=True, stop=True)
                    nc.vector.tensor_mul(QH[0:64, :], psb[0:64, :], tQ[h][:, :])
                    psb2 = pph1.tile([128, T], F32, tag="psb")
                    for n in range(2):
                        sl = slice(n * 512, (n + 1) * 512)
                        nc.tensor.matmul(psb2[0:64, sl],
                                         r(tS8[:, krow * 128:(krow + 1) * 128][:, 0:64]),
                                         r(gf8[:, sl]), start=True, stop=True)
                    nc.vector.tensor_mul(KH[0:64, :], psb2[0:64, :], tK[h][:, :])
                    # order: [space(0:64), gf*z0 (64), time (65)]
                    # q space rows are -gf*q, so row64 = -(space row 0)
                    nc.gpsimd.tensor_scalar_mul(QH[64:65, :], QH[0:1, :], -1.0)
                    nc.gpsimd.tensor_copy(out=KH[64:65, :], in_=KH[0:1, :])
                    nc.sync.dma_start(out=QH[65:66, :], in_=tim[qrow:qrow + 1, :])
                    nc.sync.dma_start(out=KH[65:66, :], in_=tim[krow:krow + 1, :])
                    BQ = hp.tile([64, T], F32R, tag="BQ")
                    psb3 = pph1.tile([128, T], F32, tag="psb")
                    for n in range(2):
                        sl = slice(n * 512, (n + 1) * 512)
                        nc.tensor.matmul(psb3[0:64, sl],
                                         r(tS4[:, h * 128:(h + 1) * 128][:, 0:64]),
                                         r(beta[:, sl]), start=True, stop=True)
                    nc.vector.tensor_mul(BQ[:], psb3[0:64, :], tQ[h][:, :])
                    AGQ = hp.tile([66, T], F32R, tag="AGQ")
                    psb4 = pph1.tile([128, T], F32, tag="psb")
                    for n in range(2):
                        sl = slice(n * 512, (n + 1) * 512)
                        nc.tensor.matmul(psb4[0:66, sl],
                                         r(tS4[:, (8 + h) * 128:(9 + h) * 128][:, 0:66]),
                                         r(ag[:, sl]), start=True, stop=True)
                    nc.vector.tensor_mul(AGQ[:], psb4[0:66, :], QH[:, :])
                    psb5 = pph1.tile([128, T], F32, tag="psb")
                    for n in range(2):
                        sl = slice(n * 512, (n + 1) * 512)
                        nc.tensor.matmul(psb5[:, sl],
                                         r(tS4[:, (4 + h) * 128:(5 + h) * 128]),
                                         r(gA[:, sl]), start=True, stop=True)
                    GAB = hp.tile([128, T], F32, tag="GAB")
                    nc.scalar.copy(out=GAB[:], in_=psb5[:])

                    for j in range(2):
                        psY = pph1.tile([65, 512], F32, tag="psY", bufs=2)
                        nsb = 4 * j + 4
                        for sb in range(nsb):
                            o = max(0, 128 * sb - 512 * j)
                            W = 512 - o
                            c0 = 512 * j + o
                            psU = pph.tile([128, 512], F32, tag="psU")
                            psM = pph.tile([128, 512], F32, tag="psM")
                            nc.tensor.matmul(
                                psM[:, o:512],
                                r(KH[:, sb * 128:(sb + 1) * 128]),
                                r(QH[:, c0:c0 + W]), start=True, stop=True)
                            nc.tensor.matmul(
                                psU[:, o:512],
                                r(tK[h][:, sb * 128:(sb + 1) * 128]),
                                r(BQ[:, c0:c0 + W]), start=True, stop=False)
                            nc.tensor.matmul(
                                psU[:, o:512],
                                r(KH[:, sb * 128:(sb + 1) * 128]),
                                r(AGQ[:, c0:c0 + W]), start=False, stop=True)
                            F = hp.tile([128, 512], F32, tag="F", bufs=3)
                            nc.scalar.activation(out=F[:, o:512], in_=psM[:, o:512],
                                                 func=AF.Tanh, scale=S_FIT,
                                                 bias=tBADJ[:])
                            G = hp.tile([128, 512], F32, tag="G", bufs=3)
                            nc.gpsimd.tensor_mul(G[:, o:512], F[:, o:512],
                                                 GAB[:, c0:c0 + W])
                            nc.vector.scalar_tensor_tensor(
                                out=G[:, o:512], in0=G[:, o:512], scalar=-1.0,
                                in1=psU[:, o:512], op0=OP.mult, op1=OP.add)
                            PT = hp.tile([128, 512], F32R, tag="PT", bufs=3)
                            nc.scalar.activation(out=PT[:, o:512], in_=G[:, o:512],
                                                 func=AF.Exp)
                            if sb >= 4 * j:
                                nc.gpsimd.tensor_mul(PT[:, o:o + 128],
                                                     PT[:, o:o + 128], tTRI[:, :])
                            nc.tensor.matmul(
                                psY[:, o:512],
                                r(tV[:, sb, h, :]),
                                r(PT[:, o:512]),
                                start=(sb == 0), stop=(sb == nsb - 1))
                        rz = hp.tile([1, 512], F32, tag="rz")
                        nc.vector.reciprocal(rz[:], psY[64:65, :])
                        cs = hp.tile([1, 512], F32R, tag="cs")
                        nc.vector.tensor_mul(cs[:], rz[:],
                                             spk[:, j * 512:(j + 1) * 512])
                        psc = pph.tile([64, 512], F32, tag="psU")
                        nc.tensor.matmul(psc[:], r(tONE[:, 0:64]), r(cs[:]),
                                         start=True, stop=True)
                        cbs = hp.tile([64, 512], F32, tag="cbs")
                        nc.scalar.copy(out=cbs[:], in_=psc[:])
                        g = h // 2
                        rows = slice((h % 2) * 64, (h % 2) * 64 + 64)
                        nc.vector.tensor_mul(tYL[g][rows, j * 512:(j + 1) * 512],
                                             psY[0:64, :], cbs[:])

                # ---- out projection: partial (T, E) ----
                for m in range(8):
                    po = pph.tile([128, 512], F32, tag="psM")
                    po2 = pph.tile([128, 512], F32, tag="psM")
                    for ne, pot in ((0, po), (1, po2)):
                        for g in range(2):
                            nc.tensor.matmul(
                                pot[:],
                                r(tYL[g][:, m * 128:(m + 1) * 128]),
                                r(tWO[:, g, ne * 512:(ne + 1) * 512]),
                                start=(g == 0), stop=(g == 1))
                    oo = hp.tile([128, T], F32, tag="oo")
                    nc.scalar.copy(out=oo[:, 0:512], in_=po[:])
                    nc.vector.tensor_copy(out=oo[:, 512:1024], in_=po2[:])
                    nc.sync.dma_start(out=out[m * 128:(m + 1) * 128, :], in_=oo[:])

    nc.finalize()
    return nc


_NC_CACHE = None


def _np_sigmoid(x):
    return 1.0 / (1.0 + np.exp(-x))


def kernel(**inputs):
    global _NC_CACHE, _S2, _S2M2, _SS, _THR, _BADJ
    x = np.asarray(inputs["x"], np.float32)
    Wqkv = np.asarray(inputs["Wqkv"], np.float32)
    bqkv = np.asarray(inputs["bqkv"], np.float32)
    Wout = np.asarray(inputs["Wout"], np.float32)
    bout = np.asarray(inputs["bout"], np.float32)
    Wimp = np.asarray(inputs["Wimp"], np.float32)
    bimp = np.asarray(inputs["bimp"], np.float32)
    Walpha = np.asarray(inputs["Walpha"], np.float32)
    balpha = np.asarray(inputs["balpha"], np.float32)
    spike_threshold = float(np.asarray(inputs["spike_threshold"]))
    log_k = np.asarray(inputs["log_k"], np.float32)
    qk_scale = float(np.asarray(inputs["qk_scale"]))

    s = _np_sigmoid(qk_scale) * 1.5
    kh = np.log1p(np.exp(log_k.astype(np.float64))) + 1e-6
    _S2 = float(s * s)
    _S2M2 = float(-2.0 * s * s)
    _SS = float(s)
    _THR = float(np.log(spike_threshold / (1.0 - spike_threshold)) - bimp[0])
    _BADJ = float(B_FIT - S_FIT)  # tanh(sf*M + (bf - sf)) = tanh(sf*(M-1)+bf)

    if _NC_CACHE is None:
        _NC_CACHE = _build()
    nc = _NC_CACHE

    onesel = np.zeros((128, 4, 16), np.float32)
    for h in range(HPC):
        onesel[0:64, h, 2 * h] = 1.0
        onesel[64:128, h, 2 * h + 1] = 1.0
        onesel[0, h, 8 + 2 * h] = 1.0
        onesel[64, h, 8 + 2 * h + 1] = 1.0
    tri = np.triu(np.ones((128, 128), np.float32))  # keep s_loc <= t_loc
    selb8 = np.zeros((8, 8, 128), np.float32)
    for i in range(8):
        selb8[i, i, :] = -1.0 if i % 2 == 0 else 1.0  # q rows negated, k rows +
    selb8 = selb8.reshape(8, 8 * 128)
    selb4 = np.zeros((4, 12, 128), np.float32)
    for i in range(4):
        selb4[i, i, :] = 1.0       # beta
        selb4[i, 4 + i, :] = 1.0   # gammaA
        selb4[i, 8 + i, :] = -1.0  # -a*gamma
    selb4 = selb4.reshape(4, 12 * 128)

    in_maps = []
    for c in range(NCORES):
        b, hg = c // 4, c % 4
        heads = list(range(HPC * hg, HPC * hg + HPC))
        qrows = np.concatenate([np.arange(h * D, (h + 1) * D) for h in heads])
        xT = np.concatenate([x[b].T, np.ones((1, T), np.float32)], 0)
        wqk_rows = np.concatenate(
            [np.concatenate([Wqkv[h * D:(h + 1) * D], Wqkv[E + h * D:E + (h + 1) * D]], 0)
             for h in heads], 0)  # (512, E)
        bqk_rows = np.concatenate(
            [np.concatenate([bqkv[h * D:(h + 1) * D], bqkv[E + h * D:E + (h + 1) * D]], 0)
             for h in heads], 0)
        wqkT = np.concatenate([wqk_rows.T, bqk_rows[None, :]], 0)  # (1025, 512)
        wv_rows = Wqkv[2 * E:][qrows]
        bv_rows = bqkv[2 * E:][qrows]
        wvT = np.concatenate([wv_rows.T, bv_rows[None, :]], 0)
        wai_rows = np.concatenate([Walpha[heads], Wimp], 0)  # (5, E)
        bai = np.concatenate([balpha[heads], np.zeros(1, np.float32)], 0)
        waiT = np.concatenate([wai_rows.T, bai[None, :]], 0)
        woT = np.ascontiguousarray(Wout[:, qrows].T)  # (256, E)
        cstv = (0.5 * bai).reshape(5, 1).astype(np.float32)
        cstav = (A_FIT / (2.0 * kh[heads])).reshape(4, 1).astype(np.float32)
        cstlv = (A_LIN / (2.0 * kh[heads])).reshape(4, 1).astype(np.float32)
        in_maps.append({
            "xT": np.ascontiguousarray(xT),
            "wqk": np.ascontiguousarray(wqkT),
            "wv": np.ascontiguousarray(wvT),
            "wai": np.ascontiguousarray(waiT),
            "wo": woT,
            "onesel": onesel,
            "cst": cstv,
            "csta": cstav,
            "cstl": cstlv,
            "tri": tri,
            "selb8": selb8,
            "selb4": selb4,
        })

    global _last_in_maps
    _last_in_maps = in_maps
    from concourse.bass_utils import run_bass_kernel_spmd
    res = run_bass_kernel_spmd(nc, in_maps, list(range(NCORES)))

    outv = np.zeros((B, T, E), np.float32)
    for c in range(NCORES):
        outv[c // 4] += res.results[c]["out"]
    outv += bout[None, None, :]
    return outv

